# revision 41
# baseline (speedup 1.0000x reference)
"""Trainium2 Bass kernel v6 for nn_NeuralAttention (cross-attention with RoPE).

Sharding: 8 cores = 4 batches (data parallel) x 2 head-groups (tensor
parallel, 8 heads each). Each core emits a partial output (its 8 heads
through the output projection); the host sums the two partials per batch
(TP unshard) and adds bo.

v6 on top of v5:
  - Host-side valid-key compaction: ~half the target positions are
    masked out (exp bias -30000 ~= 0 contribution, identical math to
    the uncompacted kernel). Keys are gathered to TP ~= 2176 columns,
    shrinking K-proj/V-proj/scores/attnV by ~47%.
  - Q/K biases folded into the PSUM->SBUF copy (per-partition scalar2
    on vector tensor_scalar) instead of contraction-1 matmuls.
  - kp/kr PSUM pools merged; the freed bank hosts the reciprocal
    broadcast matmul so pair-boundary normalization no longer blocks
    the next pair's scores (which used to share its pool).
  - Initial DMAs spread across sync/scalar/vector/gpsimd queues so the
    first Q-proj matmul starts ~4us in instead of ~17us.
  - o-proj head-0..5 partials stashed in SBUF; the tail adds heads 6,7
    without the DRAM round trip, output DMA on two queues.

All compute is bf16 (fp8 DoubleRow is ldweights-bound on trn2 and loses
to bf16 chains; plain fp8 fails the max-err budget). Inputs are shipped
host-pre-rearranged so every DMA row is >=1KB contiguous.
"""

import numpy as np
import ml_dtypes

import concourse.bass as bass
import concourse.mybir as mybir
from concourse import bacc
import concourse.tile as tile
from concourse.bass_utils import run_bass_kernel_spmd

B, L, T = 4, 512, 4096
HID, NH, HD = 1024, 16, 64
MAX_POS, BASE = 4096, 10000.0
G = 2
NHG = NH // G         # 8 heads per group
C = NHG * HD          # 512 channels per group
NCORES = 8

F32 = mybir.dt.float32
BF16 = mybir.dt.bfloat16

_BF = ml_dtypes.bfloat16


def _host_tables():
    inv_freq = 1.0 / BASE ** (np.arange(0, HD, 2, dtype=np.float32) / HD)
    t = np.arange(MAX_POS, dtype=np.float32)
    freqs = np.einsum('i,j->ij', t, inv_freq).astype(np.float32)
    emb = np.concatenate([freqs, freqs], axis=-1)
    return np.cos(emb).astype(np.float32), np.sin(emb).astype(np.float32)


def _rot_perm2():
    P = np.zeros((HD, HD), np.float32)
    for d in range(HD // 2):
        P[d, d + HD // 2] = -1.0
        P[d + HD // 2, d] = 1.0
    P2 = np.zeros((128, 128), np.float32)
    P2[:64, :64] = P
    P2[64:, 64:] = P
    return P2


def _pk(x):
    """[HID, N] -> [128, 8, N] host rearrange ((k p) n -> p k n)."""
    return np.ascontiguousarray(
        x.reshape(8, 128, x.shape[1]).transpose(1, 0, 2))


_NC_CACHE = {}


def _chunk_widths(tp):
    w = [512] * (tp // 512)
    if tp % 512:
        w.append(tp % 512)
    return w


def _slots(n_items, nt, first, last):
    """n_items distinct tile indices in [first, last]."""
    span = last - first
    out = []
    for j in range(n_items):
        s = first + (j * span) // max(n_items - 1, 1)
        while s in out:
            s += 1
        out.append(s)
    assert all(s <= last for s in out), (out, n_items, nt, first, last)
    return set(out)


def _build_nc(tp):
    nt = tp // 128          # key tiles
    W = _chunk_widths(tp)   # K' chunk widths per pair
    nch = len(W)
    coff = [sum(W[:i]) for i in range(nch)]  # chunk col offsets

    MULT = mybir.AluOpType.mult
    ADD = mybir.AluOpType.add
    EXP = mybir.ActivationFunctionType.Exp

    nc = bacc.Bacc(None, target_bir_lowering=False)

    tgtr = nc.declare_dram_parameter("tgtr", [128, 8, tp], BF16, isOutput=False)
    latr = nc.declare_dram_parameter("latr", [128, 8, L], BF16, isOutput=False)
    wqr = nc.declare_dram_parameter("wqr", [128, 8, C], BF16, isOutput=False)
    wkr = nc.declare_dram_parameter("wkr", [128, 8, C], BF16, isOutput=False)
    wvr = nc.declare_dram_parameter("wvr", [128, 8, C], BF16, isOutput=False)
    wor = nc.declare_dram_parameter("wor", [64, NHG, HID], BF16, isOutput=False)
    pt2 = nc.declare_dram_parameter("pt2", [128, 128], BF16, isOutput=False)
    cosq = nc.declare_dram_parameter("cosq", [128, L], BF16, isOutput=False)
    sinq = nc.declare_dram_parameter("sinq", [128, L], BF16, isOutput=False)
    coskg = nc.declare_dram_parameter("coskg", [128, tp], BF16, isOutput=False)
    sinkg = nc.declare_dram_parameter("sinkg", [128, tp], BF16, isOutput=False)
    bq2 = nc.declare_dram_parameter("bq2", [128, 4], F32, isOutput=False)
    bk2 = nc.declare_dram_parameter("bk2", [128, 4], F32, isOutput=False)
    bvh = nc.declare_dram_parameter("bvh", [64, NHG], F32, isOutput=False)
    mskb = nc.declare_dram_parameter("mskb", [128, nt], BF16, isOutput=False)

    outp = nc.declare_dram_parameter("out", [L, HID], F32, isOutput=True)

    with tile.TileContext(nc) as tc:
        with tc.tile_pool(name="persist", bufs=1) as persist:
            tgT = persist.tile([128, 8, tp], BF16, tag="tgT")
            kpr = [persist.tile([128, tp], BF16, tag=f"kpr{i}", name=f"kpr{i}")
                   for i in range(2)]
            qpr = [persist.tile([128, L], BF16, tag=f"qpr{i}", name=f"qpr{i}")
                   for i in range(4)]
            v16 = persist.tile([128, nt, NHG, 65], BF16, tag="v16")
            hT = persist.tile([64, NHG, L], BF16, tag="hT")
            ones_bf = persist.tile([1, 512], BF16, tag="ones_bf")
            mskb_sb = persist.tile([128, nt], BF16, tag="mskb")
            bvh_sb = persist.tile([64, NHG], F32, tag="bvh")
            bk2_sb = persist.tile([128, 4], F32, tag="bk2")
            wk_sb = persist.tile([128, 8, C], BF16, tag="wk")
            wv_sb = persist.tile([128, 8, C], BF16, tag="wv")

            nc.vector.memset(ones_bf, 1.0)

            # ---------------- pools
            scr_cm = tc.tile_pool(name="scr", bufs=2)
            scr = scr_cm.__enter__()
            avp_cm = tc.tile_pool(name="avp", bufs=1, space="PSUM")
            avp = avp_cm.__enter__()
            epool_cm = tc.tile_pool(name="epool", bufs=4)
            epool = epool_cm.__enter__()
            nrm_cm = tc.tile_pool(name="nrm", bufs=2)
            nrm = nrm_cm.__enter__()
            sps_cm = tc.tile_pool(name="sps", bufs=2, space="PSUM")
            sps = sps_cm.__enter__()
            bcp_cm = tc.tile_pool(name="bcp", bufs=1, space="PSUM")
            bcp = bcp_cm.__enter__()
            tabs_cm = tc.tile_pool(name="tabs", bufs=1)
            tabs = tabs_cm.__enter__()
            cosk_sb = tabs.tile([128, tp], BF16, tag="cosk")
            sink_sb = tabs.tile([128, tp], BF16, tag="sink")
            pt2_sb = tabs.tile([128, 128], BF16, tag="pt2")
            with tc.tile_pool(name="qin", bufs=1) as qin:
                lat_sb = qin.tile([128, 8, L], BF16, tag="lat")
                wq_sb = qin.tile([128, 8, C], BF16, tag="wq")
                cq_sb = qin.tile([128, L], BF16, tag="cq")
                sq_sb = qin.tile([128, L], BF16, tag="sq")
                bq2_sb = qin.tile([128, 4], F32, tag="bq2")
                # balanced queue layout (measured best): Q path on
                # sync+scalar, K/V bulk on gpsimd
                sl0 = slice(0, W[0])
                nc.sync.dma_start(out=lat_sb, in_=latr[:, :, :])
                nc.scalar.dma_start(out=wq_sb, in_=wqr[:, :, :])
                nc.sync.dma_start(out=bq2_sb, in_=bq2[:, :])
                nc.sync.dma_start(out=pt2_sb, in_=pt2[:, :])
                nc.sync.dma_start(out=cq_sb, in_=cosq[:, :])
                nc.sync.dma_start(out=sq_sb, in_=sinq[:, :])
                nc.gpsimd.dma_start(out=wk_sb, in_=wkr[:, :, :])
                nc.gpsimd.dma_start(out=tgT[:, :, sl0], in_=tgtr[:, :, sl0])
                nc.gpsimd.dma_start(out=bk2_sb, in_=bk2[:, :])
                nc.gpsimd.dma_start(out=cosk_sb[:, sl0], in_=coskg[:, sl0])
                nc.gpsimd.dma_start(out=sink_sb[:, sl0], in_=sinkg[:, sl0])
                nc.gpsimd.dma_start(out=wv_sb, in_=wvr[:, :, :])
                nc.gpsimd.dma_start(out=mskb_sb, in_=mskb[:, :])
                nc.gpsimd.dma_start(out=bvh_sb, in_=bvh[:, :])
                for c in range(1, nch):
                    sl = slice(coff[c], coff[c] + W[c])
                    nc.gpsimd.dma_start(out=tgT[:, :, sl], in_=tgtr[:, :, sl])
                    nc.gpsimd.dma_start(out=cosk_sb[:, sl], in_=coskg[:, sl])
                    nc.gpsimd.dma_start(out=sink_sb[:, sl], in_=sinkg[:, sl])
                # ones column for the attnV denominator; after the DMA
                # dispatches so it doesn't delay the K-path loads
                nc.gpsimd.memset(v16[:, :, :, 64:65], 1.0)

                # ---------------- Q projection + rope (PSUM via sps pool)
                for ct in range(4):
                    qp = sps.tile([128, L], F32, tag="sAB", name="qp")
                    csl = slice(ct * 128, (ct + 1) * 128)
                    for k in range(8):
                        nc.tensor.matmul(qp, wq_sb[:, k, csl],
                                         lat_sb[:, k, :],
                                         start=(k == 0), stop=(k == 7))
                    qsb = scr.tile([128, L], BF16, tag="s1", name="qsb")
                    nc.vector.tensor_scalar(
                        qsb, qp, 1.0, bq2_sb[:, ct:ct + 1], MULT, ADD)
                    qr = sps.tile([128, L], F32, tag="sAB", name="qr")
                    nc.tensor.matmul(qr, pt2_sb, qsb, start=True, stop=True)
                    t1 = scr.tile([128, L], BF16, tag="s2", name="t1")
                    nc.vector.tensor_tensor(t1, qsb, cq_sb, MULT)
                    t2 = scr.tile([128, L], BF16, tag="s3", name="t2")
                    nc.vector.tensor_tensor(t2, qr, sq_sb, MULT)
                    nc.vector.tensor_tensor(qpr[ct], t1, t2, ADD)

            # ---------------- pool for the K' chunk matmuls (kp+kr share)
            kps_cm = tc.tile_pool(name="kps", bufs=1, space="PSUM")
            kps = kps_cm.__enter__()

            def emit_kchunk(p, c):
                w = W[c]
                sl = slice(coff[c], coff[c] + w)
                csl = slice(p * 128, (p + 1) * 128)
                kp = kps.tile([128, 512], F32, tag="kp", name="kp")
                for k in range(8):
                    nc.tensor.matmul(kp[:, 0:w], wk_sb[:, k, csl],
                                     tgT[:, k, sl], start=(k == 0),
                                     stop=(k == 7))
                ksb = scr.tile([128, 512], BF16, tag="s1", name="ksb")
                nc.vector.tensor_scalar(
                    ksb[:, 0:w], kp[:, 0:w], 1.0, bk2_sb[:, p:p + 1],
                    MULT, ADD)
                kr = kps.tile([128, 512], F32, tag="kp", name="kr")
                nc.tensor.matmul(kr[:, 0:w], pt2_sb, ksb[:, 0:w],
                                 start=True, stop=True)
                t1 = scr.tile([128, 512], BF16, tag="s2", name="t1")
                nc.vector.tensor_tensor(t1[:, 0:w], ksb[:, 0:w],
                                        cosk_sb[:, sl], MULT)
                t2 = scr.tile([128, 512], BF16, tag="s3", name="t2")
                nc.vector.tensor_tensor(t2[:, 0:w], kr[:, 0:w],
                                        sink_sb[:, sl], MULT)
                nc.vector.tensor_tensor(kpr[p % 2][:, sl], t1[:, 0:w],
                                        t2[:, 0:w], ADD)

            emit_kchunk(0, 0)

            wop_ref = {}

            def emit_oproj_group(gi, heads, pool_sel):
                # gi indexes (lc, n); accumulate the given heads. Groups
                # alternate PSUM pools so consecutive groups hit different
                # banks and pipeline instead of serializing.
                lc, n = gi // 2, gi % 2
                lsl = slice(lc * 128, (lc + 1) * 128)
                nsl = slice(n * 512, (n + 1) * 512)
                if pool_sel == "ops":
                    op = ops.tile([128, 512], F32, tag="op", name="op")
                elif pool_sel == "bcp":
                    op = bcp.tile([128, 512], F32, tag="bc", name="opb")
                else:
                    op = sps.tile([128, 512], F32, tag="sAB", name="opsp")
                wo_sb = wop_ref["wo"]
                for j, h in enumerate(heads):
                    nc.tensor.matmul(op, hT[:, h, lsl], wo_sb[:, h, nsl],
                                     start=(j == 0), stop=(j == len(heads) - 1))
                return op

            def emit_norm_pre(avA, avB, hA, hB):
                # stash av (numerator rows 0..63 + denominator row 64) to
                # SBUF in one copy per head, freeing the PSUM accumulators
                # for the next pair as fast as possible
                # denominators first on DVE (feeds the recip chain);
                # numerators on the scalar engine in parallel (Copy shares
                # the Exp act-table set, so no table reload)
                work = []
                for av, h in ((avA, hA), (avB, hB)):
                    dnc = nrm.tile([1, L], F32, tag="dnc", name="dnc")
                    nc.vector.tensor_copy(out=dnc, in_=av[64:65, :])
                    osb = nrm.tile([64, L], F32, tag="osb", name="osb")
                    nc.scalar.copy(out=osb, in_=av[0:64, :])
                    work.append((osb, dnc, h))
                return work

            def emit_norm_post(work):
                # hT[h] = st[0:64]/st[64] + bv; deferred into the next
                # pair so the bc matmul never stalls the tensor stream
                for osb, dnc, h in work:
                    rf1 = nrm.tile([1, L], F32, tag="rf1", name="rf1")
                    nc.vector.reciprocal_approx_fast(out=rf1, in_=dnc)
                    rf1b = nrm.tile([1, L], BF16, tag="rf1b", name="rf1b")
                    nc.vector.tensor_copy(out=rf1b, in_=rf1)
                    # full-partition tile: tag "bc" is shared with the
                    # o-proj groups, keep allocations uniform
                    bct = bcp.tile([128, L], F32, tag="bc", name="bc")
                    bc = bct[0:64, :]
                    nc.tensor.matmul(bc, ones_bf[:, 0:64], rf1b,
                                     start=True, stop=True)
                    tmp = nrm.tile([64, L], BF16, tag="tmp", name="tmp")
                    nc.vector.tensor_tensor(tmp, osb, bc, MULT)
                    nc.vector.tensor_scalar(
                        hT[:, h, :], tmp, 1.0, bvh_sb[:, h:h + 1],
                        MULT, ADD)

            # interleave schedules
            p0_emits = [(0, c) for c in range(1, nch)] + \
                       [(1, c) for c in range(nch)]
            p0_slots = sorted(_slots(len(p0_emits), nt, 1, nt - 1))
            pk_slots = sorted(_slots(nch, nt, 2, nt - 2))
            op_slots = sorted(_slots(8, nt, 5, nt - 1))

            ops_cm = ops = None
            pending_norm = None
            for p in range(4):
                if p == 3:
                    kps_cm.__exit__(None, None, None)
                    tabs_cm.__exit__(None, None, None)
                    ops_cm = tc.tile_pool(name="ops", bufs=1, space="PSUM")
                    ops = ops_cm.__enter__()
                    wop_cm = tc.tile_pool(name="wop", bufs=1)
                    wop = wop_cm.__enter__()
                    wo_sb = wop.tile([64, NHG, HID], BF16, tag="wo")
                    nc.gpsimd.dma_start(out=wo_sb, in_=wor[:, :, :])
                    wop_ref["wo"] = wo_sb
                    fst = wop.tile([128, 8, 512], F32, tag="fst")
                    wop_ref["fst"] = fst
                hA, hB = 2 * p, 2 * p + 1
                kcur = kpr[p % 2]
                avA = avp.tile([65, L], F32, tag="avA", name="avA")
                avB = avp.tile([65, L], F32, tag="avB", name="avB")
                es = {}
                p0_q = list(p0_emits)
                pk_q = list(range(nch))
                op_q = list(range(8))
                for tt in range(nt):
                    ksl = slice(tt * 128, (tt + 1) * 128)
                    if tt == 2 and pending_norm is not None:
                        emit_norm_post(pending_norm)
                        pending_norm = None
                    if p == 0:
                        vp = sps.tile([128, C], F32, tag="sAB", name="vp")
                        for k in range(8):
                            nc.tensor.matmul(vp, tgT[:, k, ksl], wv_sb[:, k, :],
                                             start=(k == 0), stop=(k == 7))
                        nc.vector.tensor_copy(
                            out=v16[:, tt, :, 0:64],
                            in_=vp.rearrange("p (h d) -> p h d", h=NHG))
                        if tt in p0_slots and p0_q:
                            emit_kchunk(*p0_q.pop(0))
                    sAB = sps.tile([128, 2, L], F32, tag="sAB", name="sAB")
                    nc.tensor.matmul(sAB[:, 0, :], kcur[0:64, ksl],
                                     qpr[p][0:64, :], start=True, stop=True)
                    nc.tensor.matmul(sAB[:, 1, :], kcur[64:128, ksl],
                                     qpr[p][64:128, :], start=True, stop=True)
                    e16 = epool.tile([128, 2, 512], BF16, tag="e16", name="e16")
                    nc.scalar.activation(out=e16, in_=sAB, func=EXP,
                                         scale=0.125, bias=mskb_sb[:, tt:tt + 1])
                    es[tt] = e16
                    if tt > 0:
                        eP = es.pop(tt - 1)
                        nc.tensor.matmul(avA, v16[:, tt - 1, hA, :],
                                         eP[:, 0, :], start=(tt - 1 == 0),
                                         stop=False)
                        nc.tensor.matmul(avB, v16[:, tt - 1, hB, :],
                                         eP[:, 1, :], start=(tt - 1 == 0),
                                         stop=False)
                    if p in (1, 2) and tt in pk_slots and pk_q:
                        emit_kchunk(p + 1, pk_q.pop(0))
                    if p == 3 and tt in op_slots and op_q:
                        gi = op_q.pop(0)
                        op = emit_oproj_group(
                            gi, range(6), "ops" if gi % 2 == 0 else "bcp")
                        nc.scalar.copy(out=wop_ref["fst"][:, gi, :], in_=op)
                eP = es.pop(nt - 1)
                nc.tensor.matmul(avA, v16[:, nt - 1, hA, :], eP[:, 0, :],
                                 start=False, stop=True)
                nc.tensor.matmul(avB, v16[:, nt - 1, hB, :], eP[:, 1, :],
                                 start=False, stop=True)
                pending_norm = emit_norm_pre(avA, avB, hA, hB)
                if p == 3:
                    emit_norm_post(pending_norm)
                    pending_norm = None

            # ---------------- tail: heads 6,7 of the o-proj + stash add
            finp_cm = tc.tile_pool(name="finp", bufs=2)
            finp = finp_cm.__enter__()
            for gi in range(8):
                lc, n = gi // 2, gi % 2
                lsl = slice(lc * 128, (lc + 1) * 128)
                nsl = slice(n * 512, (n + 1) * 512)
                op = emit_oproj_group(gi, (6, 7),
                                      "ops" if gi % 2 == 0 else "bcp")
                osb = finp.tile([128, 512], F32, tag="fin", name="fin")
                nc.vector.tensor_tensor(osb, op, wop_ref["fst"][:, gi, :], ADD)
                eng = nc.sync if gi % 2 == 0 else nc.scalar
                eng.dma_start(out=outp[lsl, nsl], in_=osb)

            finp_cm.__exit__(None, None, None)
            wop_cm.__exit__(None, None, None)
            ops_cm.__exit__(None, None, None)
            bcp_cm.__exit__(None, None, None)
            sps_cm.__exit__(None, None, None)
            nrm_cm.__exit__(None, None, None)
            epool_cm.__exit__(None, None, None)
            avp_cm.__exit__(None, None, None)
            scr_cm.__exit__(None, None, None)

    return nc


def get_nc(tp):
    key = ("v6", tp)
    if key not in _NC_CACHE:
        nc = _build_nc(tp)
        if not nc.is_finalized():
            nc.finalize()
        _NC_CACHE[key] = nc
    return _NC_CACHE[key]


def make_in_maps(latents, target, target_mask, target_timestamp,
                 Wq, bq, Wk, bk, Wv, bv, Wo, bo):
    cos_tab, sin_tab = _host_tables()
    P2 = _rot_perm2()

    lat_ts = (np.arange(L, dtype=np.float32) * (MAX_POS - 1) / (L - 1)).astype(np.int64)
    cosq_h = np.tile(cos_tab[lat_ts].T, (2, 1)).astype(_BF)
    sinq_h = np.tile(sin_tab[lat_ts].T, (2, 1)).astype(_BF)
    pt2_h = np.ascontiguousarray(P2.T).astype(_BF)

    WoT = np.ascontiguousarray(np.asarray(Wo).T)

    # ---- valid-key compaction (masked keys contribute ~0; drop them)
    mask_np = np.asarray(target_mask).astype(bool)
    counts = mask_np.sum(axis=1)
    tp = max(512, int(-(-counts.max() // 128)) * 128)
    nt = tp // 128

    per_b = {}
    for b in range(B):
        idx = np.flatnonzero(mask_np[b])
        nv = len(idx)
        ts_full = np.asarray(target_timestamp[b]).astype(np.int64)
        ts_c = np.zeros((tp,), np.int64)
        ts_c[:nv] = ts_full[idx]
        tgt_c = np.zeros((tp, HID), np.float32)
        tgt_c[:nv] = np.asarray(target[b], np.float32)[idx]
        mcol_bias = np.full((tp,), -30000.0, np.float32)
        mcol_bias[:nv] = 0.0
        per_b[b] = {
            "tgtr": _pk(tgt_c.T).astype(_BF),
            "latr": _pk(np.asarray(latents[b]).T.astype(np.float32)).astype(_BF),
            "coskg": np.ascontiguousarray(np.tile(cos_tab[ts_c].T, (2, 1))).astype(_BF),
            "sinkg": np.ascontiguousarray(np.tile(sin_tab[ts_c].T, (2, 1))).astype(_BF),
            "mskb": np.ascontiguousarray(
                mcol_bias.reshape(nt, 128).T).astype(_BF),
        }
    per_g = {}
    for g in range(G):
        sl = slice(g * C, (g + 1) * C)
        per_g[g] = {
            "wqr": _pk(np.asarray(Wq)[sl, :].T.astype(np.float32)).astype(_BF),
            "wkr": _pk(np.asarray(Wk)[sl, :].T.astype(np.float32)).astype(_BF),
            "wvr": _pk(np.asarray(Wv)[sl, :].T.astype(np.float32)).astype(_BF),
            "wor": np.ascontiguousarray(
                WoT[sl, :].reshape(NHG, 64, HID).transpose(1, 0, 2)).astype(_BF),
            "bq2": np.ascontiguousarray(
                np.asarray(bq)[sl].reshape(4, 128).T).astype(np.float32),
            "bk2": np.ascontiguousarray(
                np.asarray(bk)[sl].reshape(4, 128).T).astype(np.float32),
            "bvh": np.ascontiguousarray(
                np.asarray(bv)[sl].reshape(NHG, 64).T).astype(np.float32),
        }

    in_maps = []
    for core in range(NCORES):
        b, g = core // 2, core % 2
        m = {"pt2": pt2_h, "cosq": cosq_h, "sinq": sinq_h}
        m.update(per_b[b])
        m.update(per_g[g])
        in_maps.append(m)
    return in_maps, tp


def kernel(latents, target, target_mask, target_timestamp,
           Wq, bq, Wk, bk, Wv, bv, Wo, bo, _trace=False, _trace_kwargs=None):
    in_maps, tp = make_in_maps(latents, target, target_mask, target_timestamp,
                               Wq, bq, Wk, bk, Wv, bv, Wo, bo)
    nc = get_nc(tp)
    res = run_bass_kernel_spmd(nc, in_maps, list(range(NCORES)),
                               trace=_trace, **(_trace_kwargs or {}))
    bo_f = np.asarray(bo, dtype=np.float32)
    full = np.zeros((B, L, HID), np.float32)
    for b in range(B):
        full[b] = res.results[2 * b]["out"] + res.results[2 * b + 1]["out"] + bo_f
    if _trace:
        return full, res
    return full


# revision 44
# speedup vs baseline: 1.1603x; 1.1603x over previous
"""Trainium2 Bass kernel v6 for nn_NeuralAttention (cross-attention with RoPE).

Sharding: 8 cores = 4 batches (data parallel) x 2 head-groups (tensor
parallel, 8 heads each). Each core emits a partial output (its 8 heads
through the output projection); the host sums the two partials per batch
(TP unshard) and adds bo.

v6 on top of v5:
  - Host-side valid-key compaction: ~half the target positions are
    masked out (exp bias -30000 ~= 0 contribution, identical math to
    the uncompacted kernel). Keys are gathered to TP ~= 2176 columns,
    shrinking K-proj/V-proj/scores/attnV by ~47%.
  - Q/K biases folded into the PSUM->SBUF copy (per-partition scalar2
    on vector tensor_scalar) instead of contraction-1 matmuls.
  - kp/kr PSUM pools merged; the freed bank hosts the reciprocal
    broadcast matmul so pair-boundary normalization no longer blocks
    the next pair's scores (which used to share its pool).
  - Initial DMAs spread across sync/scalar/vector/gpsimd queues so the
    first Q-proj matmul starts ~4us in instead of ~17us.
  - o-proj head-0..5 partials stashed in SBUF; the tail adds heads 6,7
    without the DRAM round trip, output DMA on two queues.

All compute is bf16 (fp8 DoubleRow is ldweights-bound on trn2 and loses
to bf16 chains; plain fp8 fails the max-err budget). Inputs are shipped
host-pre-rearranged so every DMA row is >=1KB contiguous.
"""

import numpy as np
import ml_dtypes

import concourse.bass as bass
import concourse.mybir as mybir
from concourse import bacc
import concourse.tile as tile
from concourse.bass_utils import run_bass_kernel_spmd

B, L, T = 4, 512, 4096
HID, NH, HD = 1024, 16, 64
MAX_POS, BASE = 4096, 10000.0
G = 2
NHG = NH // G         # 8 heads per group
C = NHG * HD          # 512 channels per group
NCORES = 8

F32 = mybir.dt.float32
BF16 = mybir.dt.bfloat16

_BF = ml_dtypes.bfloat16


def _host_tables():
    inv_freq = 1.0 / BASE ** (np.arange(0, HD, 2, dtype=np.float32) / HD)
    t = np.arange(MAX_POS, dtype=np.float32)
    freqs = np.einsum('i,j->ij', t, inv_freq).astype(np.float32)
    emb = np.concatenate([freqs, freqs], axis=-1)
    return np.cos(emb).astype(np.float32), np.sin(emb).astype(np.float32)


def _rot_perm2():
    P = np.zeros((HD, HD), np.float32)
    for d in range(HD // 2):
        P[d, d + HD // 2] = -1.0
        P[d + HD // 2, d] = 1.0
    P2 = np.zeros((128, 128), np.float32)
    P2[:64, :64] = P
    P2[64:, 64:] = P
    return P2


def _pk(x):
    """[HID, N] -> [128, 8, N] host rearrange ((k p) n -> p k n)."""
    return np.ascontiguousarray(
        x.reshape(8, 128, x.shape[1]).transpose(1, 0, 2))


_NC_CACHE = {}


def _chunk_widths(tp):
    w = [512] * (tp // 512)
    if tp % 512:
        w.append(tp % 512)
    return w


def _slots(n_items, nt, first, last):
    """n_items distinct tile indices in [first, last]."""
    span = last - first
    out = []
    for j in range(n_items):
        s = first + (j * span) // max(n_items - 1, 1)
        while s in out:
            s += 1
        out.append(s)
    assert all(s <= last for s in out), (out, n_items, nt, first, last)
    return set(out)


def _build_nc(tp):
    nt = tp // 128          # key tiles
    W = _chunk_widths(tp)   # K' chunk widths per pair
    nch = len(W)
    coff = [sum(W[:i]) for i in range(nch)]  # chunk col offsets

    MULT = mybir.AluOpType.mult
    ADD = mybir.AluOpType.add
    EXP = mybir.ActivationFunctionType.Exp

    nc = bacc.Bacc(None, target_bir_lowering=False)

    tgtr = nc.declare_dram_parameter("tgtr", [128, 8, tp], BF16, isOutput=False)
    latr = nc.declare_dram_parameter("latr", [128, 8, L], BF16, isOutput=False)
    wqr = nc.declare_dram_parameter("wqr", [128, 8, C], BF16, isOutput=False)
    wkr = nc.declare_dram_parameter("wkr", [128, 8, C], BF16, isOutput=False)
    wvr = nc.declare_dram_parameter("wvr", [128, 8, C], BF16, isOutput=False)
    wor = nc.declare_dram_parameter("wor", [64, NHG, HID], BF16, isOutput=False)
    pt2 = nc.declare_dram_parameter("pt2", [128, 128], BF16, isOutput=False)
    cosq = nc.declare_dram_parameter("cosq", [128, L], BF16, isOutput=False)
    sinq = nc.declare_dram_parameter("sinq", [128, L], BF16, isOutput=False)
    coskg = nc.declare_dram_parameter("coskg", [64, tp], BF16, isOutput=False)
    sinkg = nc.declare_dram_parameter("sinkg", [64, tp], BF16, isOutput=False)
    bq2 = nc.declare_dram_parameter("bq2", [128, 4], F32, isOutput=False)
    bk2 = nc.declare_dram_parameter("bk2", [128, 4], F32, isOutput=False)
    bvh = nc.declare_dram_parameter("bvh", [64, NHG], F32, isOutput=False)
    mskb = nc.declare_dram_parameter("mskb", [128, nt], BF16, isOutput=False)

    outp = nc.declare_dram_parameter("out", [L, HID], F32, isOutput=True)

    with tile.TileContext(nc) as tc:
        with tc.tile_pool(name="persist", bufs=1) as persist:
            tgT = persist.tile([128, 8, tp], BF16, tag="tgT")
            kpr = [persist.tile([128, tp], BF16, tag=f"kpr{i}", name=f"kpr{i}")
                   for i in range(2)]
            qpr = [persist.tile([128, L], BF16, tag=f"qpr{i}", name=f"qpr{i}")
                   for i in range(4)]
            v16 = persist.tile([128, nt, NHG, 65], BF16, tag="v16")
            hT = persist.tile([64, NHG, L], BF16, tag="hT")
            ones_bf = persist.tile([1, 512], BF16, tag="ones_bf")
            mskb_sb = persist.tile([128, nt], BF16, tag="mskb")
            bvh_sb = persist.tile([64, NHG], F32, tag="bvh")
            bk2_sb = persist.tile([128, 4], F32, tag="bk2")
            wk_sb = persist.tile([128, 8, C], BF16, tag="wk")
            wv_sb = persist.tile([128, 8, C], BF16, tag="wv")

            nc.vector.memset(ones_bf, 1.0)

            # ---------------- pools
            scr_cm = tc.tile_pool(name="scr", bufs=2)
            scr = scr_cm.__enter__()
            avp_cm = tc.tile_pool(name="avp", bufs=1, space="PSUM")
            avp = avp_cm.__enter__()
            epool_cm = tc.tile_pool(name="epool", bufs=4)
            epool = epool_cm.__enter__()
            nrm_cm = tc.tile_pool(name="nrm", bufs=2)
            nrm = nrm_cm.__enter__()
            sps_cm = tc.tile_pool(name="sps", bufs=2, space="PSUM")
            sps = sps_cm.__enter__()
            bcp_cm = tc.tile_pool(name="bcp", bufs=1, space="PSUM")
            bcp = bcp_cm.__enter__()
            tabs_cm = tc.tile_pool(name="tabs", bufs=1)
            tabs = tabs_cm.__enter__()
            cosk_sb = tabs.tile([128, tp], BF16, tag="cosk")
            sink_sb = tabs.tile([128, tp], BF16, tag="sink")
            pt2_sb = tabs.tile([128, 128], BF16, tag="pt2")
            with tc.tile_pool(name="qin", bufs=1) as qin:
                lat_sb = qin.tile([128, 8, L], BF16, tag="lat")
                wq_sb = qin.tile([128, 8, C], BF16, tag="wq")
                cq_sb = qin.tile([128, L], BF16, tag="cq")
                sq_sb = qin.tile([128, L], BF16, tag="sq")
                bq2_sb = qin.tile([128, 4], F32, tag="bq2")
                # balanced queue layout (measured best): Q path on
                # sync+scalar, K/V bulk on gpsimd
                sl0 = slice(0, W[0])
                nc.sync.dma_start(out=lat_sb, in_=latr[:, :, :])
                nc.scalar.dma_start(out=wq_sb, in_=wqr[:, :, :])
                nc.sync.dma_start(out=bq2_sb, in_=bq2[:, :])
                nc.sync.dma_start(out=pt2_sb, in_=pt2[:, :])
                nc.sync.dma_start(out=cq_sb, in_=cosq[:, :])
                nc.sync.dma_start(out=sq_sb, in_=sinq[:, :])
                # cos/sin rows 64-127 duplicate rows 0-63 (two heads share
                # dims): ship half and replicate via SBUF->SBUF DMA
                nc.gpsimd.dma_start(out=wk_sb, in_=wkr[:, :, :])
                nc.gpsimd.dma_start(out=tgT[:, :, sl0], in_=tgtr[:, :, sl0])
                nc.gpsimd.dma_start(out=bk2_sb, in_=bk2[:, :])
                nc.gpsimd.dma_start(out=cosk_sb[0:64, sl0], in_=coskg[:, sl0])
                nc.gpsimd.dma_start(out=sink_sb[0:64, sl0], in_=sinkg[:, sl0])
                nc.gpsimd.dma_start(out=cosk_sb[64:128, sl0],
                                    in_=cosk_sb[0:64, sl0])
                nc.gpsimd.dma_start(out=sink_sb[64:128, sl0],
                                    in_=sink_sb[0:64, sl0])
                nc.gpsimd.dma_start(out=wv_sb, in_=wvr[:, :, :])
                nc.gpsimd.dma_start(out=mskb_sb, in_=mskb[:, :])
                nc.gpsimd.dma_start(out=bvh_sb, in_=bvh[:, :])
                for c in range(1, nch):
                    sl = slice(coff[c], coff[c] + W[c])
                    nc.gpsimd.dma_start(out=tgT[:, :, sl], in_=tgtr[:, :, sl])
                    nc.gpsimd.dma_start(out=cosk_sb[0:64, sl], in_=coskg[:, sl])
                    nc.gpsimd.dma_start(out=sink_sb[0:64, sl], in_=sinkg[:, sl])
                    nc.gpsimd.dma_start(out=cosk_sb[64:128, sl],
                                        in_=cosk_sb[0:64, sl])
                    nc.gpsimd.dma_start(out=sink_sb[64:128, sl],
                                        in_=sink_sb[0:64, sl])
                # ones column for the attnV denominator; after the DMA
                # dispatches so it doesn't delay the K-path loads
                nc.gpsimd.memset(v16[:, :, :, 64:65], 1.0)

                # ---------------- Q projection + rope (PSUM via sps pool)
                for ct in range(4):
                    qp = sps.tile([128, L], F32, tag="sAB", name="qp")
                    csl = slice(ct * 128, (ct + 1) * 128)
                    for k in range(8):
                        nc.tensor.matmul(qp, wq_sb[:, k, csl],
                                         lat_sb[:, k, :],
                                         start=(k == 0), stop=(k == 7))
                    qsb = scr.tile([128, L], BF16, tag="s1", name="qsb")
                    nc.vector.tensor_scalar(
                        qsb, qp, 1.0, bq2_sb[:, ct:ct + 1], MULT, ADD)
                    qr = sps.tile([128, L], F32, tag="sAB", name="qr")
                    nc.tensor.matmul(qr, pt2_sb, qsb, start=True, stop=True)
                    t1 = scr.tile([128, L], BF16, tag="s2", name="t1")
                    nc.vector.tensor_tensor(t1, qsb, cq_sb, MULT)
                    t2 = scr.tile([128, L], BF16, tag="s3", name="t2")
                    nc.vector.tensor_tensor(t2, qr, sq_sb, MULT)
                    nc.vector.tensor_tensor(qpr[ct], t1, t2, ADD)

            # ---------------- pool for the K' chunk matmuls (kp+kr share)
            kps_cm = tc.tile_pool(name="kps", bufs=1, space="PSUM")
            kps = kps_cm.__enter__()

            def emit_kchunk(p, c):
                w = W[c]
                sl = slice(coff[c], coff[c] + w)
                csl = slice(p * 128, (p + 1) * 128)
                kp = kps.tile([128, 512], F32, tag="kp", name="kp")
                for k in range(8):
                    nc.tensor.matmul(kp[:, 0:w], wk_sb[:, k, csl],
                                     tgT[:, k, sl], start=(k == 0),
                                     stop=(k == 7))
                ksb = scr.tile([128, 512], BF16, tag="s1", name="ksb")
                nc.vector.tensor_scalar(
                    ksb[:, 0:w], kp[:, 0:w], 1.0, bk2_sb[:, p:p + 1],
                    MULT, ADD)
                kr = kps.tile([128, 512], F32, tag="kp", name="kr")
                nc.tensor.matmul(kr[:, 0:w], pt2_sb, ksb[:, 0:w],
                                 start=True, stop=True)
                t1 = scr.tile([128, 512], BF16, tag="s2", name="t1")
                nc.vector.tensor_tensor(t1[:, 0:w], ksb[:, 0:w],
                                        cosk_sb[:, sl], MULT)
                t2 = scr.tile([128, 512], BF16, tag="s3", name="t2")
                nc.vector.tensor_tensor(t2[:, 0:w], kr[:, 0:w],
                                        sink_sb[:, sl], MULT)
                nc.vector.tensor_tensor(kpr[p % 2][:, sl], t1[:, 0:w],
                                        t2[:, 0:w], ADD)

            emit_kchunk(0, 0)

            wop_ref = {}

            def emit_oproj_group(gi, heads, pool_sel):
                # gi indexes (lc, n); accumulate the given heads. Groups
                # alternate PSUM pools so consecutive groups hit different
                # banks and pipeline instead of serializing.
                lc, n = gi // 2, gi % 2
                lsl = slice(lc * 128, (lc + 1) * 128)
                nsl = slice(n * 512, (n + 1) * 512)
                if pool_sel == "ops":
                    op = ops.tile([128, 512], F32, tag="op", name="op")
                elif pool_sel == "bcp":
                    op = bcp.tile([128, 512], F32, tag="bc", name="opb")
                else:
                    op = sps.tile([128, 512], F32, tag="sAB", name="opsp")
                wo_sb = wop_ref["wo"]
                for j, h in enumerate(heads):
                    nc.tensor.matmul(op, hT[:, h, lsl], wo_sb[:, h, nsl],
                                     start=(j == 0), stop=(j == len(heads) - 1))
                return op

            def emit_norm_pre(avA, avB, hA, hB):
                # stash av (numerator rows 0..63 + denominator row 64) to
                # SBUF in one copy per head, freeing the PSUM accumulators
                # for the next pair as fast as possible
                # denominators first on DVE (feeds the recip chain);
                # numerators on the scalar engine in parallel (Copy shares
                # the Exp act-table set, so no table reload)
                work = []
                for av, h in ((avA, hA), (avB, hB)):
                    dnc = nrm.tile([1, L], F32, tag="dnc", name="dnc")
                    nc.vector.tensor_copy(out=dnc, in_=av[64:65, :])
                    osb = nrm.tile([64, L], F32, tag="osb", name="osb")
                    nc.scalar.copy(out=osb, in_=av[0:64, :])
                    work.append((osb, dnc, h))
                return work

            def emit_norm_post(work):
                # hT[h] = st[0:64]/st[64] + bv; deferred into the next
                # pair so the bc matmul never stalls the tensor stream
                for osb, dnc, h in work:
                    rf1 = nrm.tile([1, L], F32, tag="rf1", name="rf1")
                    nc.vector.reciprocal_approx_fast(out=rf1, in_=dnc)
                    rf1b = nrm.tile([1, L], BF16, tag="rf1b", name="rf1b")
                    nc.vector.tensor_copy(out=rf1b, in_=rf1)
                    # full-partition tile: tag "bc" is shared with the
                    # o-proj groups, keep allocations uniform
                    bct = bcp.tile([128, L], F32, tag="bc", name="bc")
                    bc = bct[0:64, :]
                    nc.tensor.matmul(bc, ones_bf[:, 0:64], rf1b,
                                     start=True, stop=True)
                    tmp = nrm.tile([64, L], BF16, tag="tmp", name="tmp")
                    nc.vector.tensor_tensor(tmp, osb, bc, MULT)
                    nc.vector.tensor_scalar(
                        hT[:, h, :], tmp, 1.0, bvh_sb[:, h:h + 1],
                        MULT, ADD)

            # interleave schedules
            p0_emits = [(0, c) for c in range(1, nch)] + \
                       [(1, c) for c in range(nch)]
            p0_slots = sorted(_slots(len(p0_emits), nt, 1, nt - 1))
            pk_slots = sorted(_slots(nch, nt, 2, nt - 2))
            op_slots = sorted(_slots(8, nt, 5, nt - 1))

            ops_cm = ops = None
            pending_norm = None
            for p in range(4):
                if p == 3:
                    kps_cm.__exit__(None, None, None)
                    tabs_cm.__exit__(None, None, None)
                    ops_cm = tc.tile_pool(name="ops", bufs=1, space="PSUM")
                    ops = ops_cm.__enter__()
                    wop_cm = tc.tile_pool(name="wop", bufs=1)
                    wop = wop_cm.__enter__()
                    wo_sb = wop.tile([64, NHG, HID], BF16, tag="wo")
                    nc.gpsimd.dma_start(out=wo_sb, in_=wor[:, :, :])
                    wop_ref["wo"] = wo_sb
                    fst = wop.tile([128, 8, 512], F32, tag="fst")
                    wop_ref["fst"] = fst
                hA, hB = 2 * p, 2 * p + 1
                kcur = kpr[p % 2]
                avA = avp.tile([65, L], F32, tag="avA", name="avA")
                avB = avp.tile([65, L], F32, tag="avB", name="avB")
                es = {}
                p0_q = list(p0_emits)
                pk_q = list(range(nch))
                op_q = list(range(8))
                for tt in range(nt):
                    ksl = slice(tt * 128, (tt + 1) * 128)
                    if tt == 2 and pending_norm is not None:
                        emit_norm_post(pending_norm)
                        pending_norm = None
                    if p == 0:
                        vp = sps.tile([128, C], F32, tag="sAB", name="vp")
                        for k in range(8):
                            nc.tensor.matmul(vp, tgT[:, k, ksl], wv_sb[:, k, :],
                                             start=(k == 0), stop=(k == 7))
                        nc.vector.tensor_copy(
                            out=v16[:, tt, :, 0:64],
                            in_=vp.rearrange("p (h d) -> p h d", h=NHG))
                        if tt in p0_slots and p0_q:
                            emit_kchunk(*p0_q.pop(0))
                    sAB = sps.tile([128, 2, L], F32, tag="sAB", name="sAB")
                    nc.tensor.matmul(sAB[:, 0, :], kcur[0:64, ksl],
                                     qpr[p][0:64, :], start=True, stop=True)
                    nc.tensor.matmul(sAB[:, 1, :], kcur[64:128, ksl],
                                     qpr[p][64:128, :], start=True, stop=True)
                    e16 = epool.tile([128, 2, 512], BF16, tag="e16", name="e16")
                    nc.scalar.activation(out=e16, in_=sAB, func=EXP,
                                         scale=0.125, bias=mskb_sb[:, tt:tt + 1])
                    es[tt] = e16
                    if tt > 0:
                        eP = es.pop(tt - 1)
                        nc.tensor.matmul(avA, v16[:, tt - 1, hA, :],
                                         eP[:, 0, :], start=(tt - 1 == 0),
                                         stop=False)
                        nc.tensor.matmul(avB, v16[:, tt - 1, hB, :],
                                         eP[:, 1, :], start=(tt - 1 == 0),
                                         stop=False)
                    if p in (1, 2) and tt in pk_slots and pk_q:
                        emit_kchunk(p + 1, pk_q.pop(0))
                    if p == 3 and tt in op_slots and op_q:
                        gi = op_q.pop(0)
                        op = emit_oproj_group(
                            gi, range(6), "ops" if gi % 2 == 0 else "bcp")
                        nc.scalar.copy(out=wop_ref["fst"][:, gi, :], in_=op)
                eP = es.pop(nt - 1)
                nc.tensor.matmul(avA, v16[:, nt - 1, hA, :], eP[:, 0, :],
                                 start=False, stop=True)
                nc.tensor.matmul(avB, v16[:, nt - 1, hB, :], eP[:, 1, :],
                                 start=False, stop=True)
                pending_norm = emit_norm_pre(avA, avB, hA, hB)
                if p == 3:
                    emit_norm_post(pending_norm)
                    pending_norm = None

            # ---------------- tail: heads 6,7 of the o-proj + stash add
            finp_cm = tc.tile_pool(name="finp", bufs=2)
            finp = finp_cm.__enter__()
            for gi in range(8):
                lc, n = gi // 2, gi % 2
                lsl = slice(lc * 128, (lc + 1) * 128)
                nsl = slice(n * 512, (n + 1) * 512)
                op = emit_oproj_group(gi, (6, 7),
                                      "ops" if gi % 2 == 0 else "bcp")
                osb = finp.tile([128, 512], F32, tag="fin", name="fin")
                nc.vector.tensor_tensor(osb, op, wop_ref["fst"][:, gi, :], ADD)
                eng = nc.sync if gi % 2 == 0 else nc.scalar
                eng.dma_start(out=outp[lsl, nsl], in_=osb)

            finp_cm.__exit__(None, None, None)
            wop_cm.__exit__(None, None, None)
            ops_cm.__exit__(None, None, None)
            bcp_cm.__exit__(None, None, None)
            sps_cm.__exit__(None, None, None)
            nrm_cm.__exit__(None, None, None)
            epool_cm.__exit__(None, None, None)
            avp_cm.__exit__(None, None, None)
            scr_cm.__exit__(None, None, None)

    return nc


def get_nc(tp):
    key = ("v6", tp)
    if key not in _NC_CACHE:
        nc = _build_nc(tp)
        if not nc.is_finalized():
            nc.finalize()
        _NC_CACHE[key] = nc
    return _NC_CACHE[key]


def make_in_maps(latents, target, target_mask, target_timestamp,
                 Wq, bq, Wk, bk, Wv, bv, Wo, bo):
    cos_tab, sin_tab = _host_tables()
    P2 = _rot_perm2()

    lat_ts = (np.arange(L, dtype=np.float32) * (MAX_POS - 1) / (L - 1)).astype(np.int64)
    cosq_h = np.tile(cos_tab[lat_ts].T, (2, 1)).astype(_BF)
    sinq_h = np.tile(sin_tab[lat_ts].T, (2, 1)).astype(_BF)
    pt2_h = np.ascontiguousarray(P2.T).astype(_BF)

    WoT = np.ascontiguousarray(np.asarray(Wo).T)

    # ---- valid-key compaction (masked keys contribute ~0; drop them)
    mask_np = np.asarray(target_mask).astype(bool)
    counts = mask_np.sum(axis=1)
    tp = max(512, int(-(-counts.max() // 128)) * 128)
    nt = tp // 128

    per_b = {}
    for b in range(B):
        idx = np.flatnonzero(mask_np[b])
        nv = len(idx)
        ts_full = np.asarray(target_timestamp[b]).astype(np.int64)
        ts_c = np.zeros((tp,), np.int64)
        ts_c[:nv] = ts_full[idx]
        tgt_c = np.zeros((tp, HID), np.float32)
        tgt_c[:nv] = np.asarray(target[b], np.float32)[idx]
        mcol_bias = np.full((tp,), -30000.0, np.float32)
        mcol_bias[:nv] = 0.0
        per_b[b] = {
            "tgtr": _pk(tgt_c.T).astype(_BF),
            "latr": _pk(np.asarray(latents[b]).T.astype(np.float32)).astype(_BF),
            "coskg": np.ascontiguousarray(cos_tab[ts_c].T).astype(_BF),
            "sinkg": np.ascontiguousarray(sin_tab[ts_c].T).astype(_BF),
            "mskb": np.ascontiguousarray(
                mcol_bias.reshape(nt, 128).T).astype(_BF),
        }
    per_g = {}
    for g in range(G):
        sl = slice(g * C, (g + 1) * C)
        per_g[g] = {
            "wqr": _pk(np.asarray(Wq)[sl, :].T.astype(np.float32)).astype(_BF),
            "wkr": _pk(np.asarray(Wk)[sl, :].T.astype(np.float32)).astype(_BF),
            "wvr": _pk(np.asarray(Wv)[sl, :].T.astype(np.float32)).astype(_BF),
            "wor": np.ascontiguousarray(
                WoT[sl, :].reshape(NHG, 64, HID).transpose(1, 0, 2)).astype(_BF),
            "bq2": np.ascontiguousarray(
                np.asarray(bq)[sl].reshape(4, 128).T).astype(np.float32),
            "bk2": np.ascontiguousarray(
                np.asarray(bk)[sl].reshape(4, 128).T).astype(np.float32),
            "bvh": np.ascontiguousarray(
                np.asarray(bv)[sl].reshape(NHG, 64).T).astype(np.float32),
        }

    in_maps = []
    for core in range(NCORES):
        b, g = core // 2, core % 2
        m = {"pt2": pt2_h, "cosq": cosq_h, "sinq": sinq_h}
        m.update(per_b[b])
        m.update(per_g[g])
        in_maps.append(m)
    return in_maps, tp


def kernel(latents, target, target_mask, target_timestamp,
           Wq, bq, Wk, bk, Wv, bv, Wo, bo, _trace=False, _trace_kwargs=None):
    in_maps, tp = make_in_maps(latents, target, target_mask, target_timestamp,
                               Wq, bq, Wk, bk, Wv, bv, Wo, bo)
    nc = get_nc(tp)
    res = run_bass_kernel_spmd(nc, in_maps, list(range(NCORES)),
                               trace=_trace, **(_trace_kwargs or {}))
    bo_f = np.asarray(bo, dtype=np.float32)
    full = np.zeros((B, L, HID), np.float32)
    for b in range(B):
        full[b] = res.results[2 * b]["out"] + res.results[2 * b + 1]["out"] + bo_f
    if _trace:
        return full, res
    return full
